# revision 7
# baseline (speedup 1.0000x reference)
"""Trainium2 Bass kernel for nn_MultiHeadAttention_51238959841967.

Head-parallel sharding over 8 NeuronCores: each core owns 2 heads x 2 batches
(4 attention matrices). Host pre-transposes activations so every device matmul
is layout-natural; an on-device AllToAll redistributes attention outputs from
head-sharding to token-sharding for the output projection + residual layernorm.

Per-core device pipeline:
  - project q,k -> [head_dim, tokens] (transposed) and v -> [tokens, head_dim]
  - scores computed transposed [key, query]; mask folded in as an fp8
    identity-matmul PSUM accumulation; exp on ScalarE with the 1/sqrt(1024)
    scale fused; softmax denominator via ones-vector matmul on TensorE
  - attention weights written (transposed) to DRAM; host transposes back
  - AllToAll -> output projection -> PE transpose -> +residual -> layernorm

Outputs per core: w_out [4, 2048, 2048] (per-pair w^T), y_out [512, 1024]
(row shard of the final normed output).
"""
import sys

for p in ("/opt/trn_rl_repo", "/opt/pypackages"):
    if p not in sys.path:
        sys.path.append(p)

import numpy as np
import ml_dtypes

NC = 8
MODEL, H, DK = 1024, 16, 64
B, S = 2, 2048
P = 128
NEG = -32768.0  # exp((s + NEG)/32) == 0 in f32; representable exactly in fp8e5


def _heads_of_core(c):
    if c < 4:
        return [4 * c, 4 * c + 2]
    return [4 * (c - 4) + 1, 4 * (c - 4) + 3]


def _cols_of_core(c):
    h0, h1 = _heads_of_core(c)
    return list(range(h0 * DK, (h0 + 1) * DK)) + list(range(h1 * DK, (h1 + 1) * DK))


_PROGRAM = None


def _build_program():
    import concourse.bacc as bacc
    import concourse.mybir as mybir
    import concourse.tile as tile
    from contextlib import ExitStack

    f32 = mybir.dt.float32
    f32r = mybir.dt.float32r
    fp8 = mybir.dt.float8e5
    AF = mybir.ActivationFunctionType
    AX = mybir.AxisListType.X

    nc = bacc.Bacc(None, num_devices=NC)

    qT = nc.dram_tensor("qT", [MODEL, B * S], f32r, kind="ExternalInput")
    kT = nc.dram_tensor("kT", [MODEL, B * S], f32r, kind="ExternalInput")
    vT = nc.dram_tensor("vT", [MODEL, B * S], f32r, kind="ExternalInput")
    mb_d = nc.dram_tensor("maskbias", [S, S], fp8, kind="ExternalInput")
    wq_d = nc.dram_tensor("wq", [MODEL, 2 * DK], f32r, kind="ExternalInput")
    bq_d = nc.dram_tensor("bq", [2 * DK], f32r, kind="ExternalInput")
    woutT_d = nc.dram_tensor("woutT", [MODEL, MODEL], f32r, kind="ExternalInput")
    bout_d = nc.dram_tensor("bout", [MODEL], f32r, kind="ExternalInput")
    gamma_d = nc.dram_tensor("gamma", [MODEL], f32, kind="ExternalInput")
    beta_d = nc.dram_tensor("beta", [MODEL], f32, kind="ExternalInput")
    resid_d = nc.dram_tensor("resid", [512, MODEL], f32, kind="ExternalInput")
    ident_d = nc.dram_tensor("ident", [P, P], f32, kind="ExternalInput")
    ones_d = nc.dram_tensor("ones_in", [P, 512], f32r, kind="ExternalInput")
    ident8_d = nc.dram_tensor("ident8", [P, P], fp8, kind="ExternalInput")

    w_out = nc.dram_tensor("w_out", [4, S, S], f32, kind="ExternalOutput")
    y_out = nc.dram_tensor("y_out", [512, MODEL], f32, kind="ExternalOutput")

    qT_r = qT.rearrange("(dc p) t -> dc p t", p=P)
    kT_r = kT.rearrange("(dc p) t -> dc p t", p=P)
    vT_r = vT.rearrange("(dc p) t -> p dc t", p=P)

    with tile.TileContext(nc) as tc, ExitStack() as ctx:
        const = ctx.enter_context(tc.tile_pool(name="const", bufs=1))
        share = ctx.enter_context(tc.tile_pool(name="share", bufs=1))
        proj = ctx.enter_context(tc.tile_pool(name="proj", bufs=1))
        actin = ctx.enter_context(tc.tile_pool(name="actin", bufs=4))
        vtp = ctx.enter_context(tc.tile_pool(name="vtp", bufs=2))
        expp = ctx.enter_context(tc.tile_pool(name="expp", bufs=17))
        wst = ctx.enter_context(tc.tile_pool(name="wst", bufs=4))
        ibcp = ctx.enter_context(tc.tile_pool(name="ibcp", bufs=2))
        invp = ctx.enter_context(tc.tile_pool(name="invp", bufs=2))
        ovp = ctx.enter_context(tc.tile_pool(name="ovp", bufs=2))
        bigp = ctx.enter_context(tc.tile_pool(name="bigp", bufs=1))
        lnp = ctx.enter_context(tc.tile_pool(name="lnp", bufs=2))
        stat = ctx.enter_context(tc.tile_pool(name="stat", bufs=6))
        psum = ctx.enter_context(tc.tile_pool(name="psum", bufs=1, space="PSUM"))
        dram = ctx.enter_context(tc.tile_pool(name="dram", bufs=1, space="DRAM"))

        # ---- constants ----
        ones = const.tile([P, 512], f32r)
        nc.sync.dma_start(ones[:], ones_d[:])
        ident_sb = const.tile([P, P], f32)
        nc.sync.dma_start(ident_sb[:], ident_d[:])
        ident8_sb = const.tile([P, P], fp8)
        nc.sync.dma_start(ident8_sb[:], ident8_d[:])
        wq_sb = const.tile([P, 8, 2 * DK], f32r)
        nc.sync.dma_start(wq_sb[:], wq_d.rearrange("(dc p) m -> p dc m", p=P))
        bq_sb = const.tile([1, 2 * DK], f32r)
        nc.sync.dma_start(bq_sb[:], bq_d[None, :])
        bout_sb = const.tile([1, MODEL], f32r)
        nc.sync.dma_start(bout_sb[:], bout_d[None, :])
        gamma_bc = const.tile([P, MODEL], f32)
        nc.sync.dma_start(gamma_bc[:], gamma_d[None, :].to_broadcast((P, MODEL)))
        beta_bc = const.tile([P, MODEL], f32)
        nc.sync.dma_start(beta_bc[:], beta_d[None, :].to_broadcast((P, MODEL)))
        eps_sb = const.tile([P, 1], f32)
        nc.vector.memset(eps_sb[:], 1e-6)
        maskbias_sb = share.tile([P, 16, S], fp8, tag="share")
        nc.sync.dma_start(maskbias_sb[:], mb_d.rearrange("(kc p) q -> p kc q", p=P))

        a2a_in = dram.tile([NC, P, 512], f32r)
        a2a_out = dram.tile([NC, P, 512], f32r)

        def r(ap):
            return ap if ap.dtype == f32r else ap.bitcast(f32r)

        def as32(ap):
            return ap.bitcast(f32) if ap.dtype == f32r else ap

        # ---- per-batch: projections + attention ----
        for b in range(B):
            qpT = proj.tile([P, S], f32r, tag="qpT")
            kpT = proj.tile([P, S], f32r, tag="kpT")
            vp = proj.tile([P, S], f32r, tag="vp")

            for name, src_r, dst in (("q", qT_r, qpT), ("k", kT_r, kpT)):
                for tb in range(4):
                    ps = psum.tile([P, 512], f32, tag="big")
                    for dc in range(8):
                        at = actin.tile([P, 512], f32r, tag="actin")
                        nc.sync.dma_start(
                            at[:], src_r[dc, :, b * S + tb * 512 : b * S + (tb + 1) * 512]
                        )
                        nc.tensor.matmul(
                            ps[:], lhsT=r(wq_sb[:, dc, :]), rhs=r(at[:]),
                            start=(dc == 0), stop=False,
                        )
                    nc.tensor.matmul(
                        ps[:], lhsT=r(bq_sb[:]), rhs=r(ones[0:1, :]),
                        start=False, stop=True,
                    )
                    nc.scalar.copy(dst[:, tb * 512 : (tb + 1) * 512], ps[:])

            for vtb in range(16):
                vt = vtp.tile([P, 8, P], f32r)
                nc.sync.dma_start(
                    vt[:], vT_r[:, :, b * S + vtb * P : b * S + (vtb + 1) * P]
                )
                ps_v = psum.tile([P, P], f32, tag="wv")
                for dc in range(8):
                    nc.tensor.matmul(
                        ps_v[:], lhsT=r(vt[:, dc, :]), rhs=r(wq_sb[:, dc, :]),
                        start=(dc == 0), stop=False,
                    )
                nc.tensor.matmul(
                    ps_v[:], lhsT=r(ones[0:1, 0:P]), rhs=r(bq_sb[:]),
                    start=False, stop=True,
                )
                nc.vector.tensor_copy(vp[:, vtb * P : (vtb + 1) * P], ps_v[:])

            # ---- attention for the 2 heads of this batch ----
            for h in range(2):
                p_idx = b * 2 + h
                hs = slice(h * DK, (h + 1) * DK)
                for qq in range(4):
                    qs = slice(qq * 512, (qq + 1) * 512)
                    wv_ps = psum.tile([DK, 512], f32, tag="wv")
                    row_ps = psum.tile([1, 512], f32, tag="row")
                    exp_tiles = []
                    for kc in range(16):
                        s_ps = psum.tile([P, 512], f32, tag="big")
                        nc.tensor.matmul(
                            s_ps[:], lhsT=ident8_sb[:], rhs=maskbias_sb[:, kc, qs],
                            start=True, stop=False, skip_group_check=True,
                        )
                        nc.tensor.matmul(
                            s_ps[:],
                            lhsT=r(kpT[hs, kc * P : (kc + 1) * P]),
                            rhs=r(qpT[hs, qs]),
                            start=False, stop=True, skip_group_check=True,
                        )
                        e_t = expp.tile([P, 512], f32r, tag="exp")
                        nc.scalar.activation(e_t[:], s_ps[:], AF.Exp, scale=1.0 / 32)
                        nc.tensor.matmul(
                            row_ps[:], lhsT=r(ones[:, 0:1]), rhs=r(e_t[:]),
                            start=(kc == 0), stop=(kc == 15), skip_group_check=True,
                        )
                        nc.tensor.matmul(
                            wv_ps[:],
                            lhsT=r(vp[:, kc * P + h * DK : kc * P + (h + 1) * DK]),
                            rhs=r(e_t[:]),
                            start=(kc == 0), stop=(kc == 15), skip_group_check=True,
                        )
                        exp_tiles.append(e_t)
                    inv_t = invp.tile([1, 512], f32)
                    nc.vector.reciprocal(out=inv_t[:], in_=row_ps[:])
                    ib_ps = psum.tile([P, 512], f32, tag="big")
                    nc.tensor.matmul(
                        ib_ps[:], lhsT=as32(ones[0:1, 0:P]), rhs=inv_t[:],
                        start=True, stop=True,
                    )
                    ibc = ibcp.tile([P, 512], f32)
                    nc.vector.tensor_copy(ibc[:], ib_ps[:])
                    ov_t = ovp.tile([DK, 512], f32r)
                    nc.vector.tensor_mul(ov_t[:], wv_ps[:], ibc[0:DK, :])
                    nc.sync.dma_start(a2a_in[b * 4 + qq, hs, :], ov_t[:])
                    for kc in range(16):
                        w_t = wst.tile([P, 512], f32, tag="wst")
                        nc.vector.tensor_mul(w_t[:], as32(exp_tiles[kc][:]), ibc[:])
                        nc.sync.dma_start(
                            w_out[p_idx, kc * P : (kc + 1) * P, qs], w_t[:]
                        )

        # ---- exchange: head-sharded -> token-sharded ----
        nc.gpsimd.collective_compute(
            "AllToAll",
            mybir.AluOpType.bypass,
            replica_groups=[list(range(NC))],
            ins=[a2a_in[:]],
            outs=[a2a_out[:]],
        )

        # ---- output projection + residual + layernorm on own 512 rows ----
        wout_sb = share.tile([P, 8, 8, P], f32r, tag="share")
        nc.sync.dma_start(wout_sb[:], woutT_d.rearrange("(g p) (f m) -> p g f m", p=P, m=P))
        a2a_sb = bigp.tile([P, 8, 512], f32r, tag="a2a")
        nc.sync.dma_start(a2a_sb[:], a2a_out[:].rearrange("g p t -> p g t"))
        resid_sb = bigp.tile([P, 4, MODEL], f32, tag="resid")
        nc.sync.dma_start(resid_sb[:], resid_d.rearrange("(tt p) F -> p tt F", p=P))
        y_sb = bigp.tile([P, 4, MODEL], f32, tag="ysb")

        for f in range(8):
            fs = slice(f * P, (f + 1) * P)
            oT_ps = psum.tile([P, 512], f32, tag="big")
            for g in range(8):
                nc.tensor.matmul(
                    oT_ps[:], lhsT=r(wout_sb[:, g, f, :]), rhs=r(a2a_sb[:, g, :]),
                    start=(g == 0), stop=False,
                )
            nc.tensor.matmul(
                oT_ps[:], lhsT=r(bout_sb[0:1, fs]), rhs=r(ones[0:1, :]),
                start=False, stop=True,
            )
            oT_sb = actin.tile([P, 512], f32, tag="actin")
            nc.scalar.copy(oT_sb[:], oT_ps[:])
            for tt in range(4):
                tr_ps = psum.tile([P, P], f32, tag="wv")
                nc.tensor.transpose(tr_ps[:], oT_sb[:, tt * P : (tt + 1) * P], ident_sb[:])
                nc.vector.tensor_add(y_sb[:, tt, fs], tr_ps[:], resid_sb[:, tt, fs])

        for tt in range(4):
            x = y_sb[:, tt, :]
            red = stat.tile([P, 1], f32, tag="stat")
            nc.vector.reduce_sum(red[:], x, axis=AX)
            nm = stat.tile([P, 1], f32, tag="stat")
            nc.scalar.mul(nm[:], red[:], -1.0 / MODEL)
            xc = lnp.tile([P, MODEL], f32, tag="ln")
            nc.scalar.add(xc[:], x, nm[:])
            sq = lnp.tile([P, MODEL], f32, tag="ln")
            nc.scalar.activation(sq[:], xc[:], AF.Square)
            var_t = stat.tile([P, 1], f32, tag="stat")
            nc.vector.reduce_sum(var_t[:], sq[:], axis=AX)
            std = stat.tile([P, 1], f32, tag="stat")
            nc.scalar.activation(std[:], var_t[:], AF.Sqrt, bias=eps_sb[:], scale=1.0 / MODEL)
            istd = stat.tile([P, 1], f32, tag="stat")
            nc.vector.reciprocal(out=istd[:], in_=std[:])
            xn = lnp.tile([P, MODEL], f32, tag="ln")
            nc.scalar.mul(xn[:], xc[:], istd[:])
            xg = lnp.tile([P, MODEL], f32, tag="ln")
            nc.vector.tensor_mul(xg[:], xn[:], gamma_bc[:])
            nc.vector.tensor_add(xg[:], xg[:], beta_bc[:])
            nc.sync.dma_start(y_out[tt * P : (tt + 1) * P, :], xg[:])

    nc.compile()
    return nc


def _get_program():
    global _PROGRAM
    if _PROGRAM is None:
        _PROGRAM = _build_program()
    return _PROGRAM


def _make_in_maps(inputs):
    q = np.ascontiguousarray(np.asarray(inputs["q"], dtype=np.float32))
    k = np.ascontiguousarray(np.asarray(inputs["k"], dtype=np.float32))
    v = np.ascontiguousarray(np.asarray(inputs["v"], dtype=np.float32))
    mask = np.asarray(inputs["mask"]).astype(bool)
    Wq = np.asarray(inputs["Wq"], dtype=np.float32)
    bq = np.asarray(inputs["bq"], dtype=np.float32)
    Wout = np.asarray(inputs["Wout"], dtype=np.float32)
    bout = np.asarray(inputs["bout"], dtype=np.float32)
    gamma = np.asarray(inputs["gamma"], dtype=np.float32)
    beta = np.asarray(inputs["beta"], dtype=np.float32)

    qf = q.reshape(B * S, MODEL)
    qT = np.ascontiguousarray(qf.T)
    kT = np.ascontiguousarray(k.reshape(B * S, MODEL).T)
    vT = np.ascontiguousarray(v.reshape(B * S, MODEL).T)

    WqT = Wq.T
    perm = np.array(sum((_cols_of_core(g) for g in range(NC)), []))
    woutT = np.ascontiguousarray(Wout.T[perm, :])
    ident = np.eye(P, dtype=np.float32)
    ident8 = np.eye(P, dtype=np.float32).astype(ml_dtypes.float8_e5m2)
    mbias = [
        np.ascontiguousarray(
            np.where(mask[p].T, np.float32(NEG), np.float32(0.0))
        ).astype(ml_dtypes.float8_e5m2)
        for p in range(2)
    ]

    in_maps = []
    for c in range(NC):
        cols = np.array(_cols_of_core(c))
        in_maps.append(
            {
                "qT": qT,
                "kT": kT,
                "vT": vT,
                "maskbias": mbias[0 if c < 4 else 1],
                "wq": np.ascontiguousarray(WqT[:, cols]),
                "bq": np.ascontiguousarray(bq[cols]),
                "woutT": woutT,
                "bout": bout,
                "gamma": gamma,
                "beta": beta,
                "resid": np.ascontiguousarray(qf[512 * c : 512 * (c + 1)]),
                "ident": ident,
                "ident8": ident8,
                "ones_in": np.ones((P, 512), np.float32),
            }
        )
    return in_maps


def run_on_hw(inputs, trace=False):
    """Returns (out, w, exec_time_ns)."""
    from concourse.bass_utils import run_bass_kernel_spmd

    nc = _get_program()
    in_maps = _make_in_maps(inputs)
    res = run_bass_kernel_spmd(nc, in_maps, list(range(NC)), trace=trace)
    results = res.results

    w = np.empty((B * H, S, S), np.float32)
    for c in range(NC):
        heads = _heads_of_core(c)
        for b in range(B):
            for hl in range(2):
                w[b * H + heads[hl]] = results[c]["w_out"][b * 2 + hl].T
    y = np.concatenate([results[c]["y_out"] for c in range(NC)], axis=0)
    out = y.reshape(B, S, MODEL)
    return out, w, res.exec_time_ns


def kernel(**inputs):
    out, w, _ = run_on_hw(inputs, trace=False)
    return out, w


# revision 14
# speedup vs baseline: 1.4262x; 1.4262x over previous
"""Trainium2 Bass kernel for nn_MultiHeadAttention_51238959841967.

Head-parallel sharding over 8 NeuronCores: each core owns 2 heads x 2 batches
(4 attention matrices). Host pre-transposes activations so every device matmul
is layout-natural; an on-device AllToAll redistributes attention outputs from
head-sharding to token-sharding for the output projection + residual layernorm.

Per-core device pipeline:
  - project q,k -> [head_dim, tokens] (transposed) and v -> [tokens, head_dim]
  - scores computed transposed [key, query]; mask folded in as an fp8
    identity-matmul PSUM accumulation; exp on ScalarE with the 1/sqrt(1024)
    scale fused; softmax denominator via ones-vector matmul on TensorE
  - attention weights written (transposed) to DRAM; host transposes back
  - AllToAll -> output projection -> PE transpose -> +residual -> layernorm

Outputs per core: w_out [4, 2048, 2048] (per-pair w^T), y_out [512, 1024]
(row shard of the final normed output).
"""
import sys

for p in ("/opt/trn_rl_repo", "/opt/pypackages"):
    if p not in sys.path:
        sys.path.append(p)

import numpy as np
import ml_dtypes

NC = 8
MODEL, H, DK = 1024, 16, 64
B, S = 2, 2048
P = 128
NEG = -32768.0  # exp((s + NEG)/32) == 0 in f32; representable exactly in fp8e5


def _heads_of_core(c):
    if c < 4:
        return [4 * c, 4 * c + 2]
    return [4 * (c - 4) + 1, 4 * (c - 4) + 3]


def _cols_of_core(c):
    h0, h1 = _heads_of_core(c)
    return list(range(h0 * DK, (h0 + 1) * DK)) + list(range(h1 * DK, (h1 + 1) * DK))


_PROGRAM = None


def _build_program():
    import concourse.bacc as bacc
    import concourse.mybir as mybir
    import concourse.tile as tile
    from contextlib import ExitStack

    f32 = mybir.dt.float32
    f32r = mybir.dt.float32r
    fp8 = mybir.dt.float8e5
    AF = mybir.ActivationFunctionType
    AX = mybir.AxisListType.X

    nc = bacc.Bacc(None, num_devices=NC)

    qT = nc.dram_tensor("qT", [MODEL, B * S], f32r, kind="ExternalInput")
    kT = nc.dram_tensor("kT", [MODEL, B * S], f32r, kind="ExternalInput")
    vT = nc.dram_tensor("vT", [MODEL, B * S], f32r, kind="ExternalInput")
    mb_d = nc.dram_tensor("maskbias", [S, S], fp8, kind="ExternalInput")
    wq_d = nc.dram_tensor("wq", [MODEL, 2 * DK], f32r, kind="ExternalInput")
    bq_d = nc.dram_tensor("bq", [2 * DK], f32r, kind="ExternalInput")
    woutT_d = nc.dram_tensor("woutT", [MODEL, MODEL], f32r, kind="ExternalInput")
    bout_d = nc.dram_tensor("bout", [MODEL], f32r, kind="ExternalInput")
    gamma_d = nc.dram_tensor("gamma", [MODEL], f32, kind="ExternalInput")
    beta_d = nc.dram_tensor("beta", [MODEL], f32, kind="ExternalInput")
    resid_d = nc.dram_tensor("resid", [512, MODEL], f32, kind="ExternalInput")
    ident_d = nc.dram_tensor("ident", [P, P], f32, kind="ExternalInput")
    ones_d = nc.dram_tensor("ones_in", [P, 512], f32r, kind="ExternalInput")
    ident8_d = nc.dram_tensor("ident8", [P, P], fp8, kind="ExternalInput")

    w_out = nc.dram_tensor("w_out", [4, S, S], f32, kind="ExternalOutput")
    y_out = nc.dram_tensor("y_out", [512, MODEL], f32, kind="ExternalOutput")

    qT_r = qT.rearrange("(dc p) t -> dc p t", p=P)
    kT_r = kT.rearrange("(dc p) t -> dc p t", p=P)
    vT_r = vT.rearrange("(dc p) t -> p dc t", p=P)

    with tile.TileContext(nc) as tc, ExitStack() as ctx:
        const = ctx.enter_context(tc.tile_pool(name="const", bufs=1))
        share = ctx.enter_context(tc.tile_pool(name="share", bufs=1))
        dram = ctx.enter_context(tc.tile_pool(name="dram", bufs=1, space="DRAM"))

        # ---- constants (whole-kernel lifetime) ----
        ones = const.tile([P, 512], f32r)
        nc.sync.dma_start(ones[:], ones_d[:])
        ident_sb = const.tile([P, P], f32)
        nc.sync.dma_start(ident_sb[:], ident_d[:])
        ident8_sb = const.tile([P, P], fp8)
        nc.sync.dma_start(ident8_sb[:], ident8_d[:])
        wq_sb = const.tile([P, 8, 2 * DK], f32r)
        nc.sync.dma_start(wq_sb[:], wq_d.rearrange("(dc p) m -> p dc m", p=P))
        bq_sb = const.tile([1, 2 * DK], f32r)
        nc.sync.dma_start(bq_sb[:], bq_d[None, :])
        bout_sb = const.tile([1, MODEL], f32r)
        nc.sync.dma_start(bout_sb[:], bout_d[None, :])
        gamma_bc = const.tile([P, MODEL], f32)
        nc.sync.dma_start(gamma_bc[:], gamma_d[None, :].to_broadcast((P, MODEL)))
        beta_bc = const.tile([P, MODEL], f32)
        nc.sync.dma_start(beta_bc[:], beta_d[None, :].to_broadcast((P, MODEL)))
        eps_sb = const.tile([P, 1], f32)
        nc.vector.memset(eps_sb[:], 1e-6)
        maskbias_sb = share.tile([P, 16, S], fp8, tag="share")
        nc.sync.dma_start(maskbias_sb[:], mb_d.rearrange("(kc p) q -> p kc q", p=P))

        a2a_in = dram.tile([NC, P, 512], f32r)
        a2a_out = dram.tile([NC, P, 512], f32r)

        def r(ap):
            return ap if ap.dtype == f32r else ap.bitcast(f32r)

        def as32(ap):
            return ap.bitcast(f32) if ap.dtype == f32r else ap

        # ================= attention scope =================
        with ExitStack() as actx:
            proj = actx.enter_context(tc.tile_pool(name="proj", bufs=2))
            actin = actx.enter_context(tc.tile_pool(name="actin", bufs=3))
            vtp = actx.enter_context(tc.tile_pool(name="vtp", bufs=2))
            expp = actx.enter_context(tc.tile_pool(name="expp", bufs=16))
            wst = actx.enter_context(tc.tile_pool(name="wst", bufs=2))
            ibcp = actx.enter_context(tc.tile_pool(name="ibcp", bufs=2))
            invp = actx.enter_context(tc.tile_pool(name="invp", bufs=1))
            ovp = actx.enter_context(tc.tile_pool(name="ovp", bufs=1))
            psA = actx.enter_context(tc.tile_pool(name="psA", bufs=2, space="PSUM"))

            for b in range(B):
                qpT = proj.tile([P, S], f32r, tag="qpT")
                kpT = proj.tile([P, S], f32r, tag="kpT")
                # vp layout per 128-token chunk kc: [h0 d0-63 | ones | h1 d0-63 | ones]
                vp = proj.tile([P, 16 * 130], f32r, tag="vp")

                for src_r, dst in ((qT_r, qpT), (kT_r, kpT)):
                    for tb in range(4):
                        ps = psA.tile([P, 512], f32, tag="big")
                        for dc in range(8):
                            at = actin.tile([P, 512], f32r, tag="actin")
                            nc.sync.dma_start(
                                at[:], src_r[dc, :, b * S + tb * 512 : b * S + (tb + 1) * 512]
                            )
                            nc.tensor.matmul(
                                ps[:], lhsT=wq_sb[:, dc, :], rhs=at[:],
                                start=(dc == 0), stop=False,
                            )
                        nc.tensor.matmul(
                            ps[:], lhsT=bq_sb[:], rhs=ones[0:1, :],
                            start=False, stop=True,
                        )
                        nc.scalar.copy(dst[:, tb * 512 : (tb + 1) * 512], ps[:])

                vp_r = vp[:].rearrange("p (kc c) -> p kc c", c=130)
                nc.vector.tensor_copy(vp_r[:, :, 64], ones[:, 0:16])
                nc.vector.tensor_copy(vp_r[:, :, 129], ones[:, 0:16])
                for vtb in range(16):
                    vt = vtp.tile([P, 8, P], f32r)
                    nc.sync.dma_start(
                        vt[:], vT_r[:, :, b * S + vtb * P : b * S + (vtb + 1) * P]
                    )
                    ps_v = psA.tile([P, P], f32, tag="wv")
                    for dc in range(8):
                        nc.tensor.matmul(
                            ps_v[:], lhsT=vt[:, dc, :], rhs=wq_sb[:, dc, :],
                            start=(dc == 0), stop=False,
                        )
                    nc.tensor.matmul(
                        ps_v[:], lhsT=ones[0:1, 0:P], rhs=bq_sb[:],
                        start=False, stop=True,
                    )
                    nc.vector.tensor_copy(vp[:, vtb * 130 : vtb * 130 + 64], ps_v[:, 0:DK])
                    nc.vector.tensor_copy(vp[:, vtb * 130 + 65 : vtb * 130 + 129], ps_v[:, DK:P])

                # ---- attention for the 2 heads of this batch ----
                for h in range(2):
                    p_idx = b * 2 + h
                    hs = slice(h * DK, (h + 1) * DK)
                    for qh in range(2):
                        q0 = qh * 1024
                        wv_ps = psA.tile([DK + 1, 1024], f32, tag="wv")
                        exp_tiles = []
                        for kc in range(16):
                            s_ps = psA.tile([P, 1024], f32, tag="big")
                            for qq in range(2):
                                nc.tensor.matmul(
                                    s_ps[:, qq * 512 : (qq + 1) * 512],
                                    lhsT=ident8_sb[:],
                                    rhs=maskbias_sb[:, kc, q0 + qq * 512 : q0 + (qq + 1) * 512],
                                    start=True, stop=False, skip_group_check=True,
                                )
                            for qq in range(2):
                                nc.tensor.matmul(
                                    s_ps[:, qq * 512 : (qq + 1) * 512],
                                    lhsT=kpT[hs, kc * P : (kc + 1) * P],
                                    rhs=qpT[hs, q0 + qq * 512 : q0 + (qq + 1) * 512],
                                    start=False, stop=True, skip_group_check=True,
                                )
                            e_t = expp.tile([P, 1024], f32r, tag="exp")
                            nc.scalar.activation(e_t[:], s_ps[:], AF.Exp, scale=1.0 / 32)
                            # w@v with a ones-column folded in: psum row DK (h0) or 0 (h1)
                            # accumulates the softmax denominator for free.
                            base = kc * 130 + (0 if h == 0 else 65)
                            for qq in range(2):
                                nc.tensor.matmul(
                                    wv_ps[:, qq * 512 : (qq + 1) * 512],
                                    lhsT=vp[:, base : base + DK + 1],
                                    rhs=e_t[:, qq * 512 : (qq + 1) * 512],
                                    start=(kc == 0), stop=(kc == 15),
                                    skip_group_check=True,
                                )
                            exp_tiles.append(e_t)
                        rs_row = DK
                        ov_lo = 0
                        inv32 = invp.tile([1, 1024], f32, tag="inv32")
                        nc.vector.reciprocal(
                            out=inv32[:], in_=wv_ps[rs_row : rs_row + 1, :]
                        )
                        invr = invp.tile([1, 1024], f32r, tag="invr")
                        nc.scalar.copy(invr[:], inv32[:])
                        ib_ps = psA.tile([P, 1024], f32, tag="big")
                        for qq in range(2):
                            nc.tensor.matmul(
                                ib_ps[:, qq * 512 : (qq + 1) * 512],
                                lhsT=ones[0:1, 0:P],
                                rhs=invr[0:1, qq * 512 : (qq + 1) * 512],
                                start=True, stop=True, skip_group_check=True,
                            )
                        ibc = ibcp.tile([P, 1024], f32)
                        nc.vector.tensor_copy(ibc[:], ib_ps[:])
                        ov_t = ovp.tile([DK, 1024], f32r)
                        nc.vector.tensor_mul(
                            ov_t[:], wv_ps[ov_lo : ov_lo + DK, :], ibc[0:DK, :]
                        )
                        j0 = b * 4 + qh * 2
                        nc.sync.dma_start(a2a_in[j0, hs, :], ov_t[:, 0:512])
                        nc.sync.dma_start(a2a_in[j0 + 1, hs, :], ov_t[:, 512:1024])
                        for kc in range(16):
                            w_t = wst.tile([P, 1024], f32, tag="wst")
                            eng = nc.gpsimd if kc % 3 == 2 else nc.vector
                            eng.tensor_mul(w_t[:], as32(exp_tiles[kc][:]), ibc[:])
                            nc.sync.dma_start(
                                w_out[p_idx, kc * P : (kc + 1) * P, q0 : q0 + 1024],
                                w_t[:],
                            )

        # ---- exchange: head-sharded -> token-sharded ----
        nc.gpsimd.collective_compute(
            "AllToAll",
            mybir.AluOpType.bypass,
            replica_groups=[list(range(NC))],
            ins=[a2a_in[:]],
            outs=[a2a_out[:]],
        )

        # ================= output projection + layernorm scope =================
        with ExitStack() as dctx:
            agp = dctx.enter_context(tc.tile_pool(name="agp", bufs=3))
            otp = dctx.enter_context(tc.tile_pool(name="otp", bufs=2))
            yp = dctx.enter_context(tc.tile_pool(name="yp", bufs=1))
            lnp = dctx.enter_context(tc.tile_pool(name="lnp", bufs=4))
            stat = dctx.enter_context(tc.tile_pool(name="stat", bufs=6))
            psD = dctx.enter_context(tc.tile_pool(name="psD", bufs=8, space="PSUM"))

            wout_sb = share.tile([P, 8, 8, P], f32r, tag="share")
            nc.sync.dma_start(
                wout_sb[:], woutT_d.rearrange("(g p) (f m) -> p g f m", p=P, m=P)
            )
            resid_sb = yp.tile([P, 4, MODEL], f32, tag="resid")
            nc.sync.dma_start(resid_sb[:], resid_d.rearrange("(tt p) F -> p tt F", p=P))
            y_sb = yp.tile([P, 4, MODEL], f32, tag="ysb")

            og = []
            for _f in range(8):
                og_t = psD.tile([P, 512], f32, tag="og")
                og.append(og_t)
            for g in range(8):
                ag = agp.tile([P, 512], f32r, tag="ag")
                nc.sync.dma_start(ag[:], a2a_out[g])
                for f in range(8):
                    nc.tensor.matmul(
                        og[f][:], lhsT=wout_sb[:, g, f, :], rhs=ag[:],
                        start=(g == 0), stop=False, skip_group_check=True,
                    )
            for f in range(8):
                nc.tensor.matmul(
                    og[f][:], lhsT=bout_sb[0:1, f * P : (f + 1) * P],
                    rhs=ones[0:1, :], start=False, stop=True, skip_group_check=True,
                )
            for f in range(8):
                fs = slice(f * P, (f + 1) * P)
                oT_sb = otp.tile([P, 512], f32, tag="ot")
                nc.scalar.copy(oT_sb[:], og[f][:])
                for tt in range(4):
                    tr_ps = psD.tile([P, P], f32, tag="og")
                    nc.tensor.transpose(
                        tr_ps[:], oT_sb[:, tt * P : (tt + 1) * P], ident_sb[:]
                    )
                    nc.vector.tensor_add(y_sb[:, tt, fs], tr_ps[:], resid_sb[:, tt, fs])

            for tt in range(4):
                x = y_sb[:, tt, :]
                red = stat.tile([P, 1], f32, tag="stat")
                nc.vector.reduce_sum(red[:], x, axis=AX)
                nm = stat.tile([P, 1], f32, tag="stat")
                nc.scalar.mul(nm[:], red[:], -1.0 / MODEL)
                xc = lnp.tile([P, MODEL], f32, tag="ln")
                nc.scalar.add(xc[:], x, nm[:])
                sq = lnp.tile([P, MODEL], f32, tag="ln")
                nc.scalar.activation(sq[:], xc[:], AF.Square)
                var_t = stat.tile([P, 1], f32, tag="stat")
                nc.vector.reduce_sum(var_t[:], sq[:], axis=AX)
                std = stat.tile([P, 1], f32, tag="stat")
                nc.scalar.activation(std[:], var_t[:], AF.Sqrt, bias=eps_sb[:], scale=1.0 / MODEL)
                istd = stat.tile([P, 1], f32, tag="stat")
                nc.vector.reciprocal(out=istd[:], in_=std[:])
                xn = lnp.tile([P, MODEL], f32, tag="ln")
                nc.scalar.mul(xn[:], xc[:], istd[:])
                xg = lnp.tile([P, MODEL], f32, tag="ln")
                nc.vector.tensor_mul(xg[:], xn[:], gamma_bc[:])
                nc.vector.tensor_add(xg[:], xg[:], beta_bc[:])
                nc.sync.dma_start(y_out[tt * P : (tt + 1) * P, :], xg[:])

    nc.compile()
    return nc


def _get_program():
    global _PROGRAM
    if _PROGRAM is None:
        _PROGRAM = _build_program()
    return _PROGRAM


def _make_in_maps(inputs):
    q = np.ascontiguousarray(np.asarray(inputs["q"], dtype=np.float32))
    k = np.ascontiguousarray(np.asarray(inputs["k"], dtype=np.float32))
    v = np.ascontiguousarray(np.asarray(inputs["v"], dtype=np.float32))
    mask = np.asarray(inputs["mask"]).astype(bool)
    Wq = np.asarray(inputs["Wq"], dtype=np.float32)
    bq = np.asarray(inputs["bq"], dtype=np.float32)
    Wout = np.asarray(inputs["Wout"], dtype=np.float32)
    bout = np.asarray(inputs["bout"], dtype=np.float32)
    gamma = np.asarray(inputs["gamma"], dtype=np.float32)
    beta = np.asarray(inputs["beta"], dtype=np.float32)

    qf = q.reshape(B * S, MODEL)
    qT = np.ascontiguousarray(qf.T)
    kT = np.ascontiguousarray(k.reshape(B * S, MODEL).T)
    vT = np.ascontiguousarray(v.reshape(B * S, MODEL).T)

    WqT = Wq.T
    perm = np.array(sum((_cols_of_core(g) for g in range(NC)), []))
    woutT = np.ascontiguousarray(Wout.T[perm, :])
    ident = np.eye(P, dtype=np.float32)
    ident8 = np.eye(P, dtype=np.float32).astype(ml_dtypes.float8_e5m2)
    mbias = [
        np.ascontiguousarray(
            np.where(mask[p].T, np.float32(NEG), np.float32(0.0))
        ).astype(ml_dtypes.float8_e5m2)
        for p in range(2)
    ]

    in_maps = []
    for c in range(NC):
        cols = np.array(_cols_of_core(c))
        in_maps.append(
            {
                "qT": qT,
                "kT": kT,
                "vT": vT,
                "maskbias": mbias[0 if c < 4 else 1],
                "wq": np.ascontiguousarray(WqT[:, cols]),
                "bq": np.ascontiguousarray(bq[cols]),
                "woutT": woutT,
                "bout": bout,
                "gamma": gamma,
                "beta": beta,
                "resid": np.ascontiguousarray(qf[512 * c : 512 * (c + 1)]),
                "ident": ident,
                "ident8": ident8,
                "ones_in": np.ones((P, 512), np.float32),
            }
        )
    return in_maps


def run_on_hw(inputs, trace=False):
    """Returns (out, w, exec_time_ns)."""
    from concourse.bass_utils import run_bass_kernel_spmd

    nc = _get_program()
    in_maps = _make_in_maps(inputs)
    res = run_bass_kernel_spmd(nc, in_maps, list(range(NC)), trace=trace)
    results = res.results

    w = np.empty((B * H, S, S), np.float32)
    for c in range(NC):
        heads = _heads_of_core(c)
        for b in range(B):
            for hl in range(2):
                w[b * H + heads[hl]] = results[c]["w_out"][b * 2 + hl].T
    y = np.concatenate([results[c]["y_out"] for c in range(NC)], axis=0)
    out = y.reshape(B, S, MODEL)
    return out, w, res.exec_time_ns


def kernel(**inputs):
    out, w, _ = run_on_hw(inputs, trace=False)
    return out, w
